# revision 1
# baseline (speedup 1.0000x reference)
"""Chamfer loss kernel for Trainium2, 8 NeuronCores, data-parallel over batch.

Math: for each batch b, point sets P (N,3) and Q (M,3):
  d2[i,j] = |p_i|^2 + |q_j|^2 - 2 p_i.q_j
  loss    = sum_b 0.5*[ sum_i sqrt(max(min_j d2,0)+eps) + sum_j sqrt(max(min_i d2,0)+eps) ]

Device strategy (per core, 4 batches):
  u[i,j] = p_i.q_j - |p_i|^2/2 - |q_j|^2/2 = -d2/2, via ONE K=13 fp16 matmul
  per tile: the fp16 hi+lo split terms hi*hi' + hi*lo' + lo*hi' and the norm
  rows are stacked as 13 contraction rows (K<=32 is free on the PE; fp16
  streams 1 cyc/row vs 4 for fp32 matmul, with ~fp32 accuracy: residual
  ~2^-22).
  Both chamfer directions are separate matmul sets (stationary side swapped),
  so every min is a free-axis reduce: min_j d2 = -2 * max_j u.
  Batch b owns PE row-strip 32b (tile_position), so paired batches' matmuls
  run concurrently in the array.
  Final tiny tiles: d2min = relu(-2*umax); dist = sqrt(d2min+eps); partial sum
  via reduce + ones-matmul; host sums the 8 per-core scalars * 0.5.
"""

import os
from contextlib import ExitStack

import numpy as np

import concourse.bass as bass
import concourse.bacc as bacc
import concourse.tile as tile
from concourse import mybir
from concourse.bass_utils import run_bass_kernel_spmd

N = 2048          # points per cloud
B_TOTAL = 32      # total batches
NCORES = 8
B_PER = B_TOTAL // NCORES   # 4 batches per core
NBLK = N // 128             # 16 stationary blocks
NCH = N // 512              # 4 moving chunks
EPS = 1e-16

F32 = mybir.dt.float32
BF16 = mybir.dt.bfloat16
F16 = mybir.dt.float16

REPEAT = int(os.environ.get("CHAMFER_REPEAT", "1"))
PS_BUFS = int(os.environ.get("CHAMFER_PS_BUFS", "2"))
ACT_SHARE = float(os.environ.get("CHAMFER_ACT_SHARE", "0.0"))
ONEDIR = int(os.environ.get("CHAMFER_ONEDIR", "0"))
ABLATE = 0


def _build_aug_bf16(ctx, tc, sb, coords_d, ones_d, norm_rows):
    """fp16 hi/lo augmented tiles, batch b at partition strip 32b, 13 rows,
    one K=13 matmul computes hi*hi' + hi*lo' + lo*hi' + norm terms:
      augX_L: [x_hi(3), x_hi(3), x_lo(3), nx_hi, nx_lo, 1, 1]
      augX_R: [x_hi(3), x_lo(3), x_hi(3), 1, 1, nx_hi, nx_lo]
    """
    nc = tc.nc
    coords_all = sb.tile([6 * B_PER, N], F32, tag="coords_all2")
    nc.sync.dma_start(coords_all[:], coords_d[:])
    c_hi = sb.tile([6 * B_PER, N], F16, tag="c_hi")
    nc.vector.tensor_copy(c_hi[:], coords_all[:])
    c_lo = sb.tile([6 * B_PER, N], F16, tag="c_lo")
    nc.vector.tensor_sub(c_lo[:], coords_all[:], c_hi[:])
    n_hi = sb.tile([2 * B_PER, N], F16, tag="n_hi")
    nc.vector.tensor_copy(n_hi[:], norm_rows[:])
    n_lo = sb.tile([2 * B_PER, N], F16, tag="n_lo")
    nc.vector.tensor_sub(n_lo[:], norm_rows[:], n_hi[:])

    tiles = {}
    for side in "pq":
        for role in "LR":
            t = sb.tile([128, N], F16, tag=f"aug_{side}_{role}",
                        name=f"aug_{side}_{role}")
            tiles[side + role] = t
    for b in range(B_PER):
        r = 32 * b
        for side in "pq":
            co = 6 * b + (0 if side == "p" else 3)
            no = 2 * b + (0 if side == "p" else 1)
            L, R = tiles[side + "L"], tiles[side + "R"]
            nc.sync.dma_start(L[r:r + 3, :], c_hi[co:co + 3, :])
            nc.sync.dma_start(L[r + 3:r + 6, :], c_hi[co:co + 3, :])
            nc.sync.dma_start(L[r + 6:r + 9, :], c_lo[co:co + 3, :])
            nc.sync.dma_start(L[r + 9:r + 10, :], n_hi[no:no + 1, :])
            nc.sync.dma_start(L[r + 10:r + 11, :], n_lo[no:no + 1, :])
            nc.sync.dma_start(L[r + 11:r + 12, :], ones_d[:])
            nc.sync.dma_start(L[r + 12:r + 13, :], ones_d[:])
            nc.sync.dma_start(R[r:r + 3, :], c_hi[co:co + 3, :])
            nc.sync.dma_start(R[r + 3:r + 6, :], c_lo[co:co + 3, :])
            nc.sync.dma_start(R[r + 6:r + 9, :], c_hi[co:co + 3, :])
            nc.sync.dma_start(R[r + 9:r + 10, :], ones_d[:])
            nc.sync.dma_start(R[r + 10:r + 11, :], ones_d[:])
            nc.sync.dma_start(R[r + 11:r + 12, :], n_hi[no:no + 1, :])
            nc.sync.dma_start(R[r + 12:r + 13, :], n_lo[no:no + 1, :])
    return tiles


def _build_body(ctx: ExitStack, tc: "tile.TileContext",
                coords_d, wsum_d, ones_d, out_d):
    nc = tc.nc

    sb = ctx.enter_context(tc.tile_pool(name="sb", bufs=1))
    pspool = ctx.enter_context(
        tc.tile_pool(name="ps", bufs=PS_BUFS, space="PSUM"))

    # ---- load inputs / norms:  -|x|^2/2 rows via matmul with block-diag -0.5
    coords_all = sb.tile([6 * B_PER, N], F32, tag="coords_all")
    nc.sync.dma_start(coords_all[:], coords_d[:])
    wsum_t = sb.tile([6 * B_PER, 2 * B_PER], F32, tag="wsum_t")
    nc.sync.dma_start(wsum_t[:], wsum_d[:])
    wsum_v = sb.tile([6 * B_PER, 2 * B_PER], F32, tag="wsum_v")
    nc.vector.tensor_copy(wsum_v[:], wsum_t[:])
    sq_all = sb.tile([6 * B_PER, N], F32, tag="sq_all")
    nc.vector.tensor_mul(sq_all[:], coords_all[:], coords_all[:])

    norm_rows = sb.tile([2 * B_PER, N], F32, tag="norm_rows")
    for h in range(2):
        hpq = pspool.tile([128, N // 2], F32, tag="ps", name=f"hpq{h}")
        for c2 in range(2):
            lo = h * 1024 + c2 * 512
            nc.tensor.matmul(hpq[0:2 * B_PER, c2 * 512:(c2 + 1) * 512],
                             wsum_v[:, :], sq_all[:, lo:lo + 512],
                             start=True, stop=True)
        nc.vector.tensor_copy(norm_rows[:, h * 1024:(h + 1) * 1024],
                              hpq[0:2 * B_PER, :])

    aug = _build_aug_bf16(ctx, tc, sb, coords_d, ones_d, norm_rows)

    # ---- main pairwise loop ----------------------------------------------
    # ONEDIR: compute u tiles once (p-blocks stationary).  Row maxes (p-side)
    # via reduce_max; col maxes (q-side) via ping-pong elementwise TT-max
    # accumulation over p-blocks, then a GPSIMD cross-partition max.
    from concourse import bass_isa
    res4 = sb.tile([128, (1 if ONEDIR else 2) * B_PER * NBLK * 2], F32,
                   tag="res4")
    conv_pool = ctx.enter_context(tc.tile_pool(name="conv", bufs=4))
    accs = []
    if ONEDIR:
        for b in range(B_PER):
            a0 = sb.tile([128, N], F32, tag=f"accA{b}", name=f"accA{b}")
            a1 = sb.tile([128, N], F32, tag=f"accB{b}", name=f"accB{b}")
            accs.append((a0, a1))
    dirs = (0,) if ONEDIR else (0, 1)
    for rep in range(REPEAT):
        for d in dirs:
            lhs_aug = aug["pL"] if d == 0 else aug["qL"]
            rhs_aug = aug["qR"] if d == 0 else aug["pR"]
            for pp in ((0, 1), (2, 3)):
                for blk in range(NBLK):
                    for half in range(2):
                        # one PSUM tile per pair: cols [0:1024]=pp[0],
                        # [1024:2048]=pp[1]; drained by ONE strided-out reduce
                        pt = pspool.tile([128, N], F32, tag="ps",
                                         name=f"pt{rep}_{d}_{blk}_{half}")
                        pts = {pp[0]: pt[:, 0:1024], pp[1]: pt[:, 1024:2048]}
                        for c2 in range(2):
                            ch = half * 2 + c2
                            for b in pp:
                                r = 32 * b
                                nc.tensor.matmul(
                                    pts[b][:, c2 * 512:(c2 + 1) * 512],
                                    lhs_aug[r:r + 13,
                                            blk * 128:(blk + 1) * 128],
                                    rhs_aug[r:r + 13,
                                            ch * 512:(ch + 1) * 512],
                                    start=True, stop=True,
                                    tile_position=(r, 0),
                                )
                        col0 = (((pp[0] * 2) + d) * NBLK + blk) * 2 + half
                        nc.vector.reduce_max(
                            res4[:, col0:col0 + 4 * NBLK + 1:4 * NBLK],
                            pt[:, 0:N if not ABLATE else 32].rearrange(
                                "p (b c) -> p b c", b=2),
                            axis=mybir.AxisListType.X)
                        for b in pp:
                            if ONEDIR:
                                col = (b * NBLK + blk) * 2 + half
                            else:
                                col = (((b * 2) + d) * NBLK + blk) * 2 + half
                            if ONEDIR:
                                hs = slice(half * 1024, (half + 1) * 1024)
                                a0, a1 = accs[b]
                                if blk == 0:
                                    nc.vector.tensor_copy(a1[:, hs],
                                                          pts[b][:, :])
                                elif blk % 2 == 1:
                                    nc.vector.tensor_max(a0[:, hs],
                                                         pts[b][:, :],
                                                         a1[:, hs])
                                else:
                                    nc.vector.tensor_max(a1[:, hs],
                                                         pts[b][:, :],
                                                         a0[:, hs])
                if ONEDIR and rep == REPEAT - 1:
                    for b in pp:
                        # NBLK even -> final col-max lives in accs[b][0]
                        nc.gpsimd.partition_all_reduce(
                            accs[b][0][:, :], accs[b][0][:, :], 128,
                            bass_isa.ReduceOp.max)

    # ---- finals: combine halves; d2 = relu(-2*umax); dist = sqrt(d2+eps)
    ncol = (1 if ONEDIR else 2) * B_PER * NBLK
    res = sb.tile([128, ncol], F32, tag="res")
    nc.vector.reduce_max(
        res[:, :], res4[:, :].rearrange("p (c h) -> p c h", h=2),
        axis=mybir.AxisListType.X)
    dd = sb.tile([128, ncol], F32, tag="dd")
    nc.scalar.activation(dd[:], res[:], mybir.ActivationFunctionType.Relu,
                         scale=-2.0)
    eps_t = sb.tile([128, 1], F32, tag="eps_t")
    nc.vector.memset(eps_t[:], EPS)
    dist = sb.tile([128, ncol], F32, tag="dist")
    nc.scalar.activation(dist[:], dd[:], mybir.ActivationFunctionType.Sqrt,
                         bias=eps_t[:, :])
    s1 = sb.tile([128, 1], F32, tag="s1")
    nc.vector.reduce_sum(s1[:], dist[:], axis=mybir.AxisListType.X)
    ones128 = sb.tile([128, 1], F32, tag="ones128")
    nc.vector.memset(ones128[:], 1.0)
    tot_ps = pspool.tile([128, N // 2], F32, tag="ps", name="tot_ps")
    if ONEDIR:
        # q-side: row 0 of each batch's all-reduced acc holds col maxes
        qrow = sb.tile([B_PER, N], F32, tag="qrow")
        for b in range(B_PER):
            nc.sync.dma_start(qrow[b:b + 1, :], accs[b][0][0:1, :])
        qdd = sb.tile([B_PER, N], F32, tag="qdd")
        nc.scalar.activation(qdd[:], qrow[:],
                             mybir.ActivationFunctionType.Relu, scale=-2.0)
        qdist = sb.tile([B_PER, N], F32, tag="qdist")
        nc.scalar.activation(qdist[:], qdd[:],
                             mybir.ActivationFunctionType.Sqrt,
                             bias=eps_t[0:B_PER, :])
        qs = sb.tile([B_PER, 1], F32, tag="qs")
        nc.vector.reduce_sum(qs[:], qdist[:], axis=mybir.AxisListType.X)
        nc.tensor.matmul(tot_ps[0:1, 0:1], s1[:, :], ones128[:, :],
                         start=True, stop=False)
        nc.tensor.matmul(tot_ps[0:1, 0:1], qs[:, :], ones128[0:B_PER, :],
                         start=False, stop=True)
    else:
        nc.tensor.matmul(tot_ps[0:1, 0:1], s1[:, :], ones128[:, :],
                         start=True, stop=True)
    tot_sb = sb.tile([1, 1], F32, tag="tot_sb")
    nc.vector.tensor_copy(tot_sb[:], tot_ps[0:1, 0:1])
    nc.sync.dma_start(out_d[:], tot_sb[:])


def build_bass() -> "bass.Bass":
    nc = bacc.Bacc("TRN2", target_bir_lowering=False, debug=False)
    coords_d = nc.declare_dram_parameter("coords", [6 * B_PER, N], F32,
                                         isOutput=False)
    wsum_d = nc.declare_dram_parameter("wsum", [6 * B_PER, 2 * B_PER], F32,
                                       isOutput=False)
    ones_d = nc.declare_dram_parameter("ones", [1, N], F16, isOutput=False)
    out_d = nc.declare_dram_parameter("out", [1, 1], F32, isOutput=True)
    with tile.TileContext(nc) as tc:
        with ExitStack() as ctx:
            _build_body(ctx, tc, coords_d, wsum_d, ones_d, out_d)
    nc.compile()
    return nc


def make_inputs(p: np.ndarray, q: np.ndarray):
    """Host-side shard/marshal: slice real part + 3-momenta, transpose to
    coordinate-major rows, stack per core."""
    import ml_dtypes
    p3 = np.ascontiguousarray(np.transpose(np.asarray(p)[0, :, :, 1:], (0, 2, 1)))
    q3 = np.ascontiguousarray(np.transpose(np.asarray(q)[:, :, 1:], (0, 2, 1)))
    wsum = np.zeros((6 * B_PER, 2 * B_PER), np.float32)
    for k in range(6 * B_PER):
        wsum[k, k // 3] = -0.5
    ones = np.ones((1, N), np.float16)
    in_maps = []
    for core in range(NCORES):
        coords = np.empty((6 * B_PER, N), np.float32)
        for b in range(B_PER):
            batch = core * B_PER + b
            coords[6 * b:6 * b + 3] = p3[batch]
            coords[6 * b + 3:6 * b + 6] = q3[batch]
        in_maps.append({"coords": coords, "wsum": wsum, "ones": ones})
    return in_maps


_NC_CACHE = None


def kernel(p: np.ndarray, q: np.ndarray) -> np.ndarray:
    global _NC_CACHE
    if _NC_CACHE is None:
        _NC_CACHE = build_bass()
    in_maps = make_inputs(p, q)
    results = run_bass_kernel_spmd(_NC_CACHE, in_maps, list(range(NCORES))).results
    total = 0.5 * float(np.sum([r["out"][0, 0] for r in results],
                               dtype=np.float64))
    return np.array(total, dtype=np.float32)

